# revision 22
# baseline (speedup 1.0000x reference)
"""Trainium2 Bass kernel for nn_Attention_31490700214694 (sparse_attention).

Pipeline per core (batch b, channel-half s; 8 cores = 4 batches x 2 halves):
  x --1x1 conv (192->288)--> qkv --dw3x3--> qkv_dw --window-attn--> attn
    --1x1 proj partial--> out-half ; host sums the two halves per batch.
The channel-half split is communication-free: qkv channels [0:288) of a
batch alias exactly to attention rows h' in [0,128) under the reference's
raw-reshape (288*65536 == 128*147456).

v4 layout (this file):
  - qkv+dw fused through SBUF in 4096-px strips (no qkv DRAM round trip).
    Strips are computed 256-px extended each side so the depthwise conv has
    its h+-1 halo locally; image edges zero-padded; w-wrap errors of the
    flat +-1 shifts subtracted post-hoc (small strided STT fixes).
  - depthwise 3x3 runs on TensorE as diag matmuls accumulated in PSUM
    (scalar_tensor_tensor on DVE measured 1x-only, so DVE chains lose);
    the 32-channel remainder chunk is packed 4 px-quarters x 32 ch onto
    128 partitions via a small DRAM bounce (plain 2D DMAs only --
    partition-split SBUF->SBUF DMA APs silently corrupt data).
  - channel-major phasing with a one-strip dw lag: phase A does chunks
    ch[0:128)+[256:288); phase B does ch[128:256) with attention groups
    g<=27 and g>=57 (whose channels are phase-A-complete) interleaved so
    DVE/ACT attention work overlaps TensorE conv work; phase C runs the
    remaining groups; phase D is the projection.  x is re-read from DRAM
    for the second pass.
  - attention per 2-row group: QK products in j-outer layout (DVE TT at 2x
    with broadcast middle dims), logits via a bf16 add-tree (tensor_reduce
    is 1x-only), exp expanded over d on ACT (broadcast-src activation),
    dense AV product on DVE at 2x, dense j-sum adds, final 1/S scale.
    GpSimd is avoided (shared SBUF port stalls DVE).
  - PSUM->SBUF copies split ACT/DVE; output partials in bf16.
"""

import os
import sys

import numpy as np

sys.path.insert(0, "/opt/trn_rl_repo")

def _install_ntff_hook():
    """Provide antenv.axon_hooks (missing in this image) so that
    run_bass_kernel_spmd(trace=True) can capture NTFF profiles."""
    import types
    import ctypes
    import contextlib

    if "antenv.axon_hooks" in sys.modules:
        return
    so_path = os.environ.get("PJRT_LIBRARY_PATH", "/opt/axon/libaxon_pjrt.so")
    try:
        lib = ctypes.CDLL(so_path)
    except OSError:
        return
    if not hasattr(lib, "axon_start_nrt_profile"):
        return
    lib.axon_start_nrt_profile.argtypes = [
        ctypes.POINTER(ctypes.c_int64), ctypes.c_size_t]
    lib.axon_start_nrt_profile.restype = ctypes.c_int64
    lib.axon_stop_nrt_profile.argtypes = [ctypes.c_char_p]
    lib.axon_stop_nrt_profile.restype = ctypes.c_int64

    @contextlib.contextmanager
    def _hook(output_dir, device_ids):
        import jax
        jax.devices()
        if device_ids:
            ids = (ctypes.c_int64 * len(device_ids))(*device_ids)
            rc = lib.axon_start_nrt_profile(ids, len(device_ids))
        else:
            rc = lib.axon_start_nrt_profile(None, 0)
        if rc != 0:
            raise RuntimeError(f"axon_start_nrt_profile rc={rc}")
        try:
            yield
        finally:
            n = lib.axon_stop_nrt_profile(str(output_dir).encode())
            if n < 0:
                raise RuntimeError(f"axon_stop_nrt_profile rc={n}")

    mod = types.ModuleType("antenv.axon_hooks")
    mod.get_axon_ntff_profile_hook = lambda: _hook
    mod.set_axon_ntff_profile_hook = lambda h: None
    sys.modules["antenv.axon_hooks"] = mod
    import antenv
    antenv.axon_hooks = mod


_install_ntff_hook()

import concourse.bass as bass
import concourse.tile as tile
from concourse import bacc, mybir
import concourse.bass_utils as _bu
from concourse.bass_utils import run_bass_kernel_spmd

_bu.upload_artifacts = lambda tmpdir: tmpdir

F32 = mybir.dt.float32
BF16 = mybir.dt.bfloat16

C_IN = 192
C_QKV = 288
C_ATTN = 96
NPX = 65536
HP = 128            # attention h'-rows per core
WP = 256
CTOK = 576
SCALE = 8 ** (-0.5)

STRIP = 4096
NSTRIP = NPX // STRIP           # 16
EXT = STRIP + 512               # 4608 (256 halo each side)
GW = 2                          # guard columns each side
BW = EXT + 2 * GW               # 4612 qkv strip buffer width
NT = 512

# dw engine knobs: for each strip, True -> TensorE diag, False -> DVE STT.
DW0_PE = [True] * NSTRIP        # chunk0 during phase A
DW1_PE = [True] * NSTRIP        # chunk1 during phase B
DW2_PE = True                   # packed remainder chunk
# attention knobs
QK_ON_GP = 0      # QK product on GpSimd when g % QK_ON_GP == 1 (0 = never)
U2_ON_GP = 0      # second j-sum add on GpSimd when g % U2_ON_GP == 1

# tap order: t = (dh+1)*3 + (dwc+1), offsets in flat shuffled px space
TAP_OFF = [dh * 256 + dwc for dh in (-1, 0, 1) for dwc in (-1, 0, 1)]

Add = mybir.AluOpType.add
Mult = mybir.AluOpType.mult
AX = mybir.AxisListType.X
Copy = mybir.ActivationFunctionType.Copy
Exp = mybir.ActivationFunctionType.Exp

_CACHE = {}


def _shuffle_perm(block=4):
    h = w = 256
    idx = np.arange(h * w).reshape(1, 1, h, w)
    x = np.transpose(idx, (0, 2, 3, 1)).reshape(1, h * w, 1)
    x = x.reshape(1, block, h // block, block, w // block, 1)
    x = np.transpose(x, (0, 2, 4, 1, 3, 5)).reshape(1, h * w, 1)
    return x.reshape(h * w).copy()


def _shuffle_back_perm(block=4):
    h = w = 256
    idx = np.arange(h * w).reshape(1, 1, h, w)
    x = np.transpose(idx, (0, 2, 3, 1)).reshape(1, h * w, 1)
    x = x.reshape(1, h // block, w // block, block, block, 1)
    x = np.transpose(x, (0, 3, 1, 4, 2, 5)).reshape(1, h * w, 1)
    return x.reshape(h * w).copy()


def _load_x_ext(nc, xpool, xs, s):
    """Load x for extended strip s: px [s*4096-256, s*4096+4352) with
    zero pads outside the image."""
    x0 = xpool.tile([128, EXT], BF16, tag="x0")
    x1 = xpool.tile([64, EXT], BF16, tag="x1")
    p0 = s * STRIP - 256
    lo = max(0, p0)
    hi = min(NPX, p0 + EXT)
    d0 = lo - p0
    d1 = hi - p0
    if d0 > 0:
        nc.vector.memset(x0[:, 0:d0], 0.0)
        nc.vector.memset(x1[:, 0:d0], 0.0)
    if d1 < EXT:
        nc.vector.memset(x0[:, d1:EXT], 0.0)
        nc.vector.memset(x1[:, d1:EXT], 0.0)
    nc.sync.dma_start(x0[:, d0:d1], xs[0:128, lo:hi])
    nc.sync.dma_start(x1[:, d0:d1], xs[128:192, lo:hi])
    return x0, x1


def _emit_qkv_chunk(nc, qpool, ppool, wq0, wq1, x0, x1, m0, m1, tag):
    """qkv for channel slice [m0,m1) over the extended strip -> SBUF tile
    [m1-m0, BW] with 2 guard cols each side (zeroed)."""
    mm = m1 - m0
    qt = qpool.tile([mm, BW], BF16, tag=tag)
    nc.vector.memset(qt[:, 0:GW], 0.0)
    nc.vector.memset(qt[:, GW + EXT:BW], 0.0)
    PW = 1536
    for nb in range(EXT // PW):          # 3 tiles of 1536
        ps = ppool.tile([128, PW], F32, tag="qk")
        for j in range(PW // NT):
            sl = slice(j * NT, (j + 1) * NT)
            xsl = slice(nb * PW + j * NT, nb * PW + (j + 1) * NT)
            nc.tensor.matmul(ps[:mm, sl], wq0[:, m0:m1], x0[:, xsl],
                             start=True, stop=False)
            nc.tensor.matmul(ps[:mm, sl], wq1[:, m0:m1], x1[:, xsl],
                             start=False, stop=True)
        if nb == 1:
            nc.vector.tensor_copy(
                qt[:, GW + nb * PW:GW + (nb + 1) * PW], ps[:mm, :])
        else:
            nc.scalar.copy(qt[:, GW + nb * PW:GW + (nb + 1) * PW], ps[:mm, :])
    return qt


def _emit_dw_fixes(nc, dwo, src, wneg, base, row_stride, nrows):
    """Subtract wrong w-wrap contributions at out columns 0 and 255.
    dwo: [128, nrows*256] acc (viewed), src: buffer the taps read,
    base: index of output px 0's center in src."""
    d3 = dwo[:].rearrange("p (r w) -> p r w", w=256)
    for dh in (-1, 0, 1):
        tL = (dh + 1) * 3 + 0
        tR = (dh + 1) * 3 + 2
        oL = base + dh * 256 - 1
        sL = src[:, oL:oL + (nrows - 1) * row_stride + 1:row_stride]
        nc.vector.scalar_tensor_tensor(
            d3[:, :, 0:1], sL.unsqueeze(2), wneg[:, tL:tL + 1],
            d3[:, :, 0:1], op0=Mult, op1=Add)
        oR = base + dh * 256 + 256
        sR = src[:, oR:oR + (nrows - 1) * row_stride + 1:row_stride]
        nc.vector.scalar_tensor_tensor(
            d3[:, :, 255:256], sR.unsqueeze(2), wneg[:, tR:tR + 1],
            d3[:, :, 255:256], op0=Mult, op1=Add)


def _emit_dw_pe(nc, dpool, ppool, wd, qt, wneg, qkv_dw, c0, cw, s):
    """dw for a 128(cw)-channel chunk on TensorE diag matmuls."""
    dwo = dpool.tile([cw, STRIP], BF16, tag="dwo")
    for nb in range(STRIP // NT):        # 8 out-tiles
        ps = ppool.tile([128, NT], F32, tag="dw")
        for t in range(9):
            off = GW + 256 + nb * NT + TAP_OFF[t]
            nc.tensor.matmul(ps[:cw, :], wd[:, bass.ts(t, 128)][:, :cw],
                             qt[:, off:off + NT],
                             start=(t == 0), stop=(t == 8))
        if nb % 2 == 0:
            nc.scalar.copy(dwo[:, bass.ts(nb, NT)], ps[:cw, :])
        else:
            nc.vector.tensor_copy(dwo[:, bass.ts(nb, NT)], ps[:cw, :])
    _emit_dw_fixes(nc, dwo, qt, wneg, GW + 256, 256, 16)
    nc.sync.dma_start(qkv_dw[c0:c0 + cw, s * STRIP:(s + 1) * STRIP], dwo[:])


def _emit_dw_dve(nc, dpool, wdw, qt, wneg, qkv_dw, c0, cw, s):
    """dw for a chunk on DVE: +1-shifted copy for 2x alignment, then
    1 mul + 8 STT."""
    tl = dpool.tile([cw, BW], BF16, tag="tl")
    nc.vector.tensor_copy(tl[:, 0:BW - 2], qt[:, 1:BW - 1])
    dwo = dpool.tile([cw, STRIP], BF16, tag="dwo")
    first = True
    for t in range(9):
        off = GW + 256 + TAP_OFF[t]
        if TAP_OFF[t] % 2 != 0:
            src = tl[:, off - 1:off - 1 + STRIP]
        else:
            src = qt[:, off:off + STRIP]
        if first:
            nc.vector.tensor_scalar_mul(dwo[:], src, wdw[:, t:t + 1])
            first = False
        else:
            nc.vector.scalar_tensor_tensor(
                dwo[:], src, wdw[:, t:t + 1], dwo[:], op0=Mult, op1=Add)
    _emit_dw_fixes(nc, dwo, qt, wneg, GW + 256, 256, 16)
    nc.sync.dma_start(qkv_dw[c0:c0 + cw, s * STRIP:(s + 1) * STRIP], dwo[:])


def _emit_dw_m2(nc, dpool, ppool, wd2, wdw2, wneg2, qt2, qs2, qkv_dw, s):
    """dw for the 32-channel remainder: pack 4 px-quarters onto 128
    partitions (partition = 32*q + c) via SBUF->SBUF DMA, then PE diag
    (or DVE mul/add chain)."""
    PKW = 1544                     # 1540 used + pad
    nc.scalar.dma_start(qs2[:, 0:BW], qt2[:])
    pk = dpool.tile([128, PKW], BF16, tag="pk2")
    nc.vector.memset(pk[:, 1540:PKW], 0.0)
    for q in range(4):
        nc.sync.dma_start(pk[32 * q:32 * q + 32, 0:1540],
                          qs2[:, q * 1024:q * 1024 + 1540])
    dwo = dpool.tile([128, 1024], BF16, tag="dwo2")
    if DW2_PE:
        for nb in range(2):
            ps = ppool.tile([128, NT], F32, tag="dw")
            for t in range(9):
                off = 258 + nb * NT + TAP_OFF[t]
                nc.tensor.matmul(ps[:], wd2[:, bass.ts(t, 128)],
                                 pk[:, off:off + NT],
                                 start=(t == 0), stop=(t == 8))
            if nb % 2 == 0:
                nc.scalar.copy(dwo[:, bass.ts(nb, NT)], ps[:])
            else:
                nc.vector.tensor_copy(dwo[:, bass.ts(nb, NT)], ps[:])
    else:
        tmp = dpool.tile([128, 1024], BF16, tag="tmp2")
        first = True
        for t in range(9):
            off = 258 + TAP_OFF[t]
            src = pk[:, off:off + 1024]
            if first:
                nc.vector.tensor_scalar_mul(dwo[:], src, wdw2[:, t:t + 1])
                first = False
            else:
                nc.vector.tensor_scalar_mul(tmp[:], src, wdw2[:, t:t + 1])
                nc.vector.tensor_tensor(dwo[:], dwo[:], tmp[:], op=Add)
    _emit_dw_fixes(nc, dwo, pk, wneg2, 258, 256, 4)
    for q in range(4):
        nc.sync.dma_start(
            qkv_dw[256:288,
                   s * STRIP + q * 1024:s * STRIP + (q + 1) * 1024],
            dwo[32 * q:32 * q + 32, :])


def _emit_attn_group(ctx, nc, apool, qv, av, g):
    """Window attention for h'-row pair g (tokens (2g+dh, 2j+dw))."""
    T = apool.tile([128, 4 * CTOK], BF16, tag="T")
    src = qv[2 * g:2 * g + 2].rearrange("dh (j dw) c -> j dh dw c", dw=2)
    nc.sync.dma_start(
        T[:].rearrange("p (dh dw c) -> p dh dw c", dh=2, dw=2), src)
    t3 = T[:].rearrange("p (tok c) -> p tok c", tok=4)

    # P[p, (jk, iq, h, d)] = k[jk] * q[iq]  (j-outer)
    P = apool.tile([128, 3072], BF16, tag="P")
    p4 = P[:].rearrange("p (j i c) -> p j i c", j=4, i=4)
    k_b = t3[:, :, 192:384].unsqueeze(2).broadcast_to([128, 4, 4, 192])
    q_b = t3[:, :, 0:192].unsqueeze(1).broadcast_to([128, 4, 4, 192])
    if QK_ON_GP and g % QK_ON_GP == 1:
        nc.gpsimd.tensor_tensor(p4, k_b, q_b, op=Mult)
    else:
        nc.vector.tensor_tensor(p4, k_b, q_b, op=Mult)

    # L[p, (j,i,h)] via bf16 add tree over d=24 (8+8+8 then 8->1)
    p24 = P[:].rearrange("p (g d) -> p g d", d=24)
    s8 = apool.tile([128, 1024], BF16, tag="s8")
    s8v = s8[:].rearrange("p (g d) -> p g d", d=8)
    nc.vector.tensor_tensor(s8v, p24[:, :, 0:8], p24[:, :, 8:16], op=Add)
    nc.vector.tensor_tensor(s8v, s8v, p24[:, :, 16:24], op=Add)
    s4 = apool.tile([128, 512], BF16, tag="s4")
    s4v = s4[:].rearrange("p (g d) -> p g d", d=4)
    nc.vector.tensor_tensor(s4v, s8v[:, :, 0:4], s8v[:, :, 4:8], op=Add)
    s2 = apool.tile([128, 256], BF16, tag="s2")
    s2v = s2[:].rearrange("p (g d) -> p g d", d=2)
    nc.vector.tensor_tensor(s2v, s4v[:, :, 0:2], s4v[:, :, 2:4], op=Add)
    L = apool.tile([128, 128], BF16, tag="L")
    Lv = L[:].unsqueeze(2)
    nc.vector.tensor_tensor(Lv, s2v[:, :, 0:1], s2v[:, :, 1:2], op=Add)

    # EEx = exp(SCALE*L) expanded over d on ACT (proven broadcast pattern)
    EEx = apool.tile([128, 3072], BF16, tag="EEx")
    nc.scalar.activation(
        EEx[:].rearrange("p (g d) -> p g d", d=24),
        L[:].unsqueeze(2).broadcast_to([128, 128, 24]), Exp,
        scale=float(SCALE))
    # S = sum_j exp (from d==0 slice); R = 1/S
    S = apool.tile([128, 32], F32, tag="S")
    e4 = EEx[:].rearrange("p (j i h d) -> p j i h d", j=4, i=4, h=8)
    nc.vector.tensor_reduce(
        S[:].rearrange("p (i h) -> p i h", i=4),
        e4[:, :, :, :, 0].rearrange("p j i h -> p i h j"), axis=AX, op=Add)
    R = apool.tile([128, 32], F32, tag="R")
    nc.vector.reciprocal(R[:], S[:])

    # P2 = EEx * v (bcast i), dense 2x
    P2 = apool.tile([128, 3072], BF16, tag="P2")
    v_b = t3[:, :, 384:576].unsqueeze(2).broadcast_to([128, 4, 4, 192])
    nc.vector.tensor_tensor(
        P2[:].rearrange("p (j i c) -> p j i c", j=4, i=4),
        EEx[:].rearrange("p (j i c) -> p j i c", j=4, i=4), v_b, op=Mult)

    # U[p, (i,h,d)] = sum_j P2 (dense adds); O = U * R (bcast over d)
    U1 = apool.tile([128, 768], BF16, tag="U1")
    U2 = apool.tile([128, 768], BF16, tag="U2")
    O = apool.tile([128, 768], BF16, tag="O")
    nc.vector.tensor_tensor(U1[:], P2[:, 0:768], P2[:, 768:1536], op=Add)
    nc.vector.tensor_tensor(U2[:], P2[:, 1536:2304], P2[:, 2304:3072], op=Add)
    nc.vector.tensor_tensor(U1[:], U1[:], U2[:], op=Add)
    r_b = R[:].rearrange("p (i h) -> p i h", i=4).unsqueeze(3).broadcast_to(
        [128, 4, 8, 24])
    nc.vector.tensor_tensor(
        O[:].rearrange("p (i h d) -> p i h d", i=4, h=8),
        U1[:].rearrange("p (i h d) -> p i h d", i=4, h=8), r_b, op=Mult)

    dst = av[2 * g:2 * g + 2].rearrange("dh (j dw) c -> j dh dw c", dw=2)
    nc.sync.dma_start(
        dst, O[:].rearrange("p (dh dw c) -> p dh dw c", dh=2, dw=2))


def _emit_proj(ctx, tc, nc, attn_dram, wpT, out_dram, ppool):
    """partial out[192, NPX] = wpT.T @ attn[96, NPX], bf16 out."""
    xpool = ctx.enter_context(tc.tile_pool(name="pj_x", bufs=2))
    opool = ctx.enter_context(tc.tile_pool(name="pj_o", bufs=2))
    PW = 1536
    blocks = [(i * PW, PW) for i in range(NPX // PW)]
    rem = NPX - (NPX // PW) * PW
    if rem:
        blocks.append((NPX - rem, rem))
    for p0, w in blocks:
        at = xpool.tile([C_ATTN, PW], BF16, tag="at")
        nc.sync.dma_start(at[:, 0:w], attn_dram[:, p0:p0 + w])
        o0 = opool.tile([128, PW], BF16, tag="o0")
        o1 = opool.tile([64, PW], BF16, tag="o1")
        ps0 = ppool.tile([128, PW], F32, tag="qk")
        ps1 = ppool.tile([128, PW], F32, tag="qk")
        for j in range(0, w, NT):
            sl = slice(j, j + NT)
            nc.tensor.matmul(ps0[:, sl], wpT[:, 0:128], at[:, sl],
                             start=True, stop=True)
            nc.tensor.matmul(ps1[:64, sl], wpT[:, 128:192], at[:, sl],
                             start=True, stop=True)
        nc.vector.tensor_copy(o0[:, 0:w], ps0[:, 0:w])
        nc.vector.tensor_copy(o1[:, 0:w], ps1[:64, 0:w])
        nc.sync.dma_start(out_dram[0:128, p0:p0 + w], o0[:, 0:w])
        nc.sync.dma_start(out_dram[128:192, p0:p0 + w], o1[:, 0:w])


def _build():
    if "nc" in _CACHE:
        return _CACHE["nc"]
    nc = bacc.Bacc("TRN2", target_bir_lowering=False, debug=False,
                   num_devices=8)
    xs = nc.dram_tensor("xs", [C_IN, NPX], BF16, kind="ExternalInput").ap()
    wqkvT = nc.dram_tensor("wqkvT", [C_IN, C_QKV], BF16,
                           kind="ExternalInput").ap()
    wdw = nc.dram_tensor("wdw", [3, 128, 9], F32, kind="ExternalInput").ap()
    wdwn = nc.dram_tensor("wdwn", [3, 128, 9], F32, kind="ExternalInput").ap()
    wdiag = nc.dram_tensor("wdiag", [3, 9, 128, 128], BF16,
                           kind="ExternalInput").ap()
    wprojT = nc.dram_tensor("wprojT", [C_ATTN, 192], BF16,
                            kind="ExternalInput").ap()
    out = nc.dram_tensor("out", [192, NPX], BF16, kind="ExternalOutput").ap()

    if os.environ.get("KERNEL_DBG"):
        qkv_dw = nc.dram_tensor("qkv_dw_out", [C_QKV, NPX], BF16,
                                kind="ExternalOutput").ap()
        attn_dram = nc.dram_tensor("attn_out", [C_ATTN, NPX], BF16,
                                   kind="ExternalOutput").ap()
    else:
        qkv_dw = nc.dram_tensor("qkv_dw_buf", [C_QKV, NPX], BF16).ap()
        attn_dram = nc.dram_tensor("attn_buf", [C_ATTN, NPX], BF16).ap()
    qs2 = nc.dram_tensor("qs2_buf", [32, BW], BF16).ap()

    qv = qkv_dw.rearrange("c p -> (c p)").rearrange(
        "(hh ww cc) -> hh ww cc", ww=WP, cc=CTOK)
    av = attn_dram.rearrange("c p -> (c p)").rearrange(
        "(hh ww cc) -> hh ww cc", ww=WP, cc=192)

    from contextlib import ExitStack
    with tile.TileContext(nc) as tc:
        with ExitStack() as ctx:
            ppool = ctx.enter_context(
                tc.tile_pool(name="psum", bufs=2, space="PSUM"))
            wpool = ctx.enter_context(tc.tile_pool(name="wts", bufs=1))
            xpool = ctx.enter_context(tc.tile_pool(name="xin", bufs=2))
            qpool = ctx.enter_context(tc.tile_pool(name="qkv", bufs=2))
            q2pool = ctx.enter_context(tc.tile_pool(name="qkv2", bufs=3))
            dpool = ctx.enter_context(tc.tile_pool(name="dw", bufs=2))
            apool = ctx.enter_context(tc.tile_pool(name="attn", bufs=2))

            # --- weights (resident) ---
            wq0 = wpool.tile([128, C_QKV], BF16, tag="wq0")
            wq1 = wpool.tile([64, C_QKV], BF16, tag="wq1")
            nc.sync.dma_start(wq0[:], wqkvT[0:128, :])
            nc.sync.dma_start(wq1[:], wqkvT[128:192, :])
            wdw_t = []
            wdwn_t = []
            for ci in range(3):
                wt = wpool.tile([128, 9], F32, tag=f"wdw{ci}")
                nc.sync.dma_start(wt[:], wdw[ci])
                wdw_t.append(wt)
                wn = wpool.tile([128, 9], F32, tag=f"wdwn{ci}")
                nc.sync.dma_start(wn[:], wdwn[ci])
                wdwn_t.append(wn)
            wd_t = []
            for ci in range(3):
                wd = wpool.tile([128, 9 * 128], BF16, tag=f"wd{ci}")
                nc.sync.dma_start(
                    wd[:].rearrange("p (t m) -> p t m", t=9),
                    wdiag[ci].rearrange("t k m -> k t m"))
                wd_t.append(wd)
            wpT = wpool.tile([C_ATTN, 192], BF16, tag="wp")
            nc.sync.dma_start(wpT[:], wprojT[:, :])

            # --- phase A: chunks 0+2 (ch 0:128, 256:288) qkv+dw, all
            #     strips, dw lagging one strip so PE never waits on the
            #     PSUM->SBUF copies ---
            def _dw_a(qt, qt2, s):
                if DW0_PE[s]:
                    _emit_dw_pe(nc, dpool, ppool, wd_t[0], qt, wdwn_t[0],
                                qkv_dw, 0, 128, s)
                else:
                    _emit_dw_dve(nc, dpool, wdw_t[0], qt, wdwn_t[0],
                                 qkv_dw, 0, 128, s)
                _emit_dw_m2(nc, dpool, ppool, wd_t[2], wdw_t[2], wdwn_t[2],
                            qt2, qs2, qkv_dw, s)

            prevA = None
            for s in range(NSTRIP):
                x0, x1 = _load_x_ext(nc, xpool, xs, s)
                qt = _emit_qkv_chunk(nc, qpool, ppool, wq0, wq1, x0, x1,
                                     0, 128, "q")
                qt2 = _emit_qkv_chunk(nc, q2pool, ppool, wq0, wq1, x0, x1,
                                      256, 288, "q2")
                if prevA is not None:
                    _dw_a(*prevA)
                prevA = (qt, qt2, s)
            _dw_a(*prevA)

            # --- phase B: chunk1 over all strips; attention groups whose
            #     channels live entirely in chunks 0/2 interleave here:
            #     g<=27 (ch<126.5) and g>=57 (ch>=256.5) ---
            gB = list(range(28)) + list(range(57, 64))
            gi = 0

            def _dw_b(qt1, s):
                if DW1_PE[s]:
                    _emit_dw_pe(nc, dpool, ppool, wd_t[1], qt1, wdwn_t[1],
                                qkv_dw, 128, 128, s)
                else:
                    _emit_dw_dve(nc, dpool, wdw_t[1], qt1, wdwn_t[1],
                                 qkv_dw, 128, 128, s)

            prevB = None
            for s in range(NSTRIP):
                x0, x1 = _load_x_ext(nc, xpool, xs, s)
                qt1 = _emit_qkv_chunk(nc, qpool, ppool, wq0, wq1, x0, x1,
                                      128, 256, "q")
                if prevB is not None:
                    _dw_b(*prevB)
                prevB = (qt1, s)
                g_hi = (len(gB) * (s + 1)) // NSTRIP
                while gi < g_hi:
                    _emit_attn_group(ctx, nc, apool, qv, av, gB[gi])
                    gi += 1
            _dw_b(*prevB)

            # --- phase C: remaining attention groups ---
            for g in range(28, 57):
                _emit_attn_group(ctx, nc, apool, qv, av, g)

            # --- phase D: proj ---
            _emit_proj(ctx, tc, nc, attn_dram, wpT, out, ppool)
    nc.compile()
    _CACHE["nc"] = nc
    return nc


def kernel(x, w_qkv, w_dw, w_proj, shuffle):
    import ml_dtypes
    bf = ml_dtypes.bfloat16
    x = np.asarray(x, dtype=np.float32)
    w_qkv = np.asarray(w_qkv, dtype=np.float32)
    w_dw = np.asarray(w_dw, dtype=np.float32)
    w_proj = np.asarray(w_proj, dtype=np.float32)
    do_shuffle = bool(int(np.asarray(shuffle)))

    B = x.shape[0]
    xf = x.reshape(B, C_IN, NPX)
    if do_shuffle:
        xf = xf[:, :, _shuffle_perm()]

    wq = w_qkv[:, :, 0, 0]                      # [576, 192]
    wqT = np.ascontiguousarray(wq.T)            # [192, 576]
    wdw_f = w_dw[:, 0].reshape(576, 9)          # [576, 9]
    wp = w_proj[:, :, 0, 0]                     # [192, 192]

    in_maps = []
    for b in range(B):
        for s in range(2):
            wdw_h = wdw_f[s * C_QKV:(s + 1) * C_QKV]      # [288, 9]
            wdw_in = np.zeros((3, 128, 9), dtype=np.float32)
            wdw_in[0] = wdw_h[0:128]
            wdw_in[1] = wdw_h[128:256]
            # packed remainder: partition 32*q + c -> channel 256 + c
            wdw_in[2] = np.tile(wdw_h[256:288], (4, 1))
            wdiag = np.zeros((3, 9, 128, 128), dtype=bf)
            for ci in range(3):
                for t in range(9):
                    wdiag[ci, t][np.arange(128), np.arange(128)] = \
                        wdw_in[ci, :, t].astype(bf)
            in_maps.append({
                "xs": np.ascontiguousarray(xf[b]).astype(bf),
                "wqkvT": np.ascontiguousarray(
                    wqT[:, s * C_QKV:(s + 1) * C_QKV]).astype(bf),
                "wdw": wdw_in,
                "wdwn": -wdw_in,
                "wdiag": wdiag,
                "wprojT": np.ascontiguousarray(
                    wp[:, s * C_ATTN:(s + 1) * C_ATTN].T).astype(bf),
            })

    nc = _build()
    res = run_bass_kernel_spmd(nc, in_maps, core_ids=list(range(8)),
                               trace=bool(int(os.environ.get("KERNEL_TRACE", "0"))))
    _CACHE["last_results"] = res

    outs = [res.results[i]["out"] for i in range(8)]
    of = np.stack([outs[2 * b].astype(np.float32) + outs[2 * b + 1].astype(np.float32)
                   for b in range(B)])
    if do_shuffle:
        of = of[:, :, _shuffle_back_perm()]
    return of.reshape(B, 192, 256, 256).astype(np.float32)
